# revision 1
# baseline (speedup 1.0000x reference)
"""Multi-head self-attention (B=2, S=2048, D=1024, H=16, causal) on 8 TRN2 cores.

Sharding: core c handles batch b=c//4 and head-group g=c%4 (4 heads each).
Host pre-transposes x and the weight slices so the kernel never needs an
on-chip transpose:
  xT   [1024, 2048] = x[b].T
  wqT/wkT/wvT [1024, 256] = W.T[:, g*256:(g+1)*256]
  woT  [256, 1024] = Wo[:, g*256:(g+1)*256].T
Host sums the 4 per-group partial outputs per batch at the end.

On-chip dataflow per core (all matmul dtypes fp32r by default):
  qT/kT [256, 2048] (head dim on partitions), v [2048, 4*65] (with a ones
  column appended per head so the PV matmul also accumulates the softmax
  denominator in psum row 64).  Scores are computed transposed
  (scoresT[j, i]) so softmax needs no transpose at all; there is no
  max-subtraction (scores are O(+-6), exp is safe in fp32).
"""

import os
import sys

sys.path.insert(0, "/opt/trn_rl_repo")
os.environ.setdefault("MYCRO_LOCAL_CACHE", "1")

import numpy as np

import concourse.bacc as bacc
import concourse.bass as bass
import concourse.mybir as mybir
import concourse.tile as tile
from concourse import bass_utils

# The agent image's antenv lacks axon_hooks, so bass_utils' trace path dies on
# import.  Register a shim module that lazily builds the ctypes NTFF hook.
if "antenv.axon_hooks" not in sys.modules:
    import types

    _shim = types.ModuleType("antenv.axon_hooks")
    _shim._HOOK = None

    def _set_hook(hook, _m=_shim):
        _m._HOOK = hook

    def _get_hook(_m=_shim):
        if _m._HOOK is None:
            try:
                from trn_agent_boot.trn_boot import _ntff_profile_via_ctypes

                _m._HOOK = _ntff_profile_via_ctypes("/opt/axon/libaxon_pjrt.so")
            except Exception:
                _m._HOOK = None
        return _m._HOOK

    _shim.set_axon_ntff_profile_hook = _set_hook
    _shim.get_axon_ntff_profile_hook = _get_hook
    sys.modules["antenv.axon_hooks"] = _shim

B, S, D, H = 2, 2048, 1024, 16
DK = 64                      # head dim
HC = 4                       # heads per core
GC = HC * DK                 # 256 cols per head-group
N_CORES = 8
SCALE = 1.0 / np.sqrt(DK)    # 0.125

F32 = mybir.dt.float32
MM_DT = getattr(mybir.dt, os.environ.get("BASS_MM_DT", "float32r"))

TRACE = False
LAST_RESULTS = None


def _dram(ap):
    """Bitcast a DRAM fp32 AP to the matmul dtype for DMA into fp32r tiles."""
    if MM_DT == F32:
        return ap
    return ap.bitcast(MM_DT)


def build_bass():
    nc = bacc.Bacc("TRN2", target_bir_lowering=False, debug=False)

    xT_d = nc.dram_tensor("xT", [D, S], F32, kind="ExternalInput")
    wqT_d = nc.dram_tensor("wqT", [D, GC], F32, kind="ExternalInput")
    wkT_d = nc.dram_tensor("wkT", [D, GC], F32, kind="ExternalInput")
    wvT_d = nc.dram_tensor("wvT", [D, GC], F32, kind="ExternalInput")
    woT_d = nc.dram_tensor("woT", [GC, D], F32, kind="ExternalInput")
    mask_d = nc.dram_tensor("mask", [128, 4, 512], F32, kind="ExternalInput")
    out_d = nc.dram_tensor("out", [S, D], F32, kind="ExternalOutput")

    EXP = mybir.ActivationFunctionType.Exp

    with tile.TileContext(nc) as tc:
        with (
            nc.allow_low_precision(reason="fp32r tiles carry full fp32 storage"),
            tc.tile_pool(name="const", bufs=1) as const,
            tc.tile_pool(name="work", bufs=3) as work,
            tc.tile_pool(name="apool", bufs=2) as apool,
            tc.tile_pool(name="opool", bufs=2) as opool,
            tc.tile_pool(name="rpool", bufs=2) as rpool,
            tc.tile_pool(name="psmm", bufs=3, space="PSUM") as psmm,
            tc.tile_pool(name="psout", bufs=2, space="PSUM") as psout,
        ):
            # ---- load inputs -------------------------------------------------
            xT_dr = _dram(xT_d.rearrange("(o p) s -> p o s", p=128))
            xts = []
            for ko in range(8):
                xt = const.tile([128, S], MM_DT, name=f"xt{ko}")
                nc.sync.dma_start(xt[:], xT_dr[:, ko, :])
                xts.append(xt)
            wq = const.tile([128, 8, GC], MM_DT)
            nc.gpsimd.dma_start(wq[:], _dram(wqT_d.rearrange("(o p) m -> p o m", p=128)))
            # descriptor generation for the strided weight loads is slow; put
            # them on the gpsimd queue so they don't serialize behind xT/wq
            wk = const.tile([128, 8, GC], MM_DT)
            nc.gpsimd.dma_start(wk[:], _dram(wkT_d.rearrange("(o p) m -> p o m", p=128)))
            wv = const.tile([128, 8, GC], MM_DT)
            nc.gpsimd.dma_start(wv[:], _dram(wvT_d.rearrange("(o p) m -> p o m", p=128)))
            wo = const.tile([128, 2, D], MM_DT)
            nc.gpsimd.dma_start(wo[:], _dram(woT_d.rearrange("(o p) n -> p o n", p=128)))
            maskt = const.tile([128, 4, 512], F32)
            nc.gpsimd.dma_start(maskt[:], mask_d[:])

            ones_f = const.tile([128, 64], F32)
            nc.vector.memset(ones_f[:], 1.0)
            ones64 = const.tile([1, 64], MM_DT)
            nc.vector.tensor_copy(ones64[:], ones_f[0:1, :])

            # ---- projections -------------------------------------------------
            # qT/kT: per (head-pair mo, s-half sbh) tiles [128, 1024] so the
            # attention phase can start before all projections finish
            qts = [[const.tile([128, 1024], MM_DT, name=f"q{m}{s}")
                    for s in range(2)] for m in range(2)]
            kts = [[const.tile([128, 1024], MM_DT, name=f"k{m}{s}")
                    for s in range(2)] for m in range(2)]
            # v: per j-chunk tiles; per head: 64 value cols + 1 ones col
            vts = []
            for io in range(16):
                vt = const.tile([128, HC * 65], MM_DT, name=f"v{io}")
                nc.vector.tensor_copy(
                    vt.rearrange("p (h u) -> p h u", u=65)[:, :, 64],
                    ones_f[:, 0:4],
                )
                vts.append(vt)

            for w_sb, dst in ((wq, qts), (wk, kts)):
                for mo in range(2):
                    for sbh in range(2):
                        # one [128,2,512] psum; ko outer so the stationary
                        # weight is reused by the two sb matmuls (1 LDW / 2 MM)
                        ps = psmm.tile([128, 2, 512], F32, tag="mm")
                        for ko in range(8):
                            for sb2 in range(2):
                                sb = 2 * sbh + sb2
                                nc.tensor.matmul(
                                    ps[:, sb2, :],
                                    (w_sb[:, ko, mo * 128:(mo + 1) * 128]),
                                    (xts[ko][:, sb * 512:(sb + 1) * 512]),
                                    start=(ko == 0),
                                    stop=(ko == 7),
                                    skip_group_check=True,
                                )
                        nc.vector.tensor_copy(
                            dst[mo][sbh][:],
                            ps.rearrange("p a n -> p (a n)"),
                        )

            for io in range(16):
                ps = psmm.tile([128, 256], F32, tag="mm")
                for ko in range(8):
                    nc.tensor.matmul(
                        ps[:],
                        (xts[ko][:, io * 128:(io + 1) * 128]),
                        (wv[:, ko, :]),
                        start=(ko == 0),
                        stop=(ko == 7),
                    )
                nc.vector.tensor_copy(
                    vts[io].rearrange("p (h u) -> p h u", u=65)[:, :, 0:64],
                    ps.rearrange("p (h e) -> p h e", e=64),
                )

            # ---- attention + output projection, per 512-query block ---------
            for Q in range(4):
                i0 = Q * 512
                aT = apool.tile([128, 2, 512], MM_DT, tag="aT")
                for mo in range(2):
                    nchunks = (Q + 1) * 4
                    out_ps = [
                        psout.tile([65, 512], F32, tag="out", name=f"out_ps{_h}")
                        for _h in range(2)
                    ]
                    for jc in range(nchunks):
                        sc = psmm.tile([128, 2, 512], F32, tag="mm")
                        for hp in range(2):
                            nc.tensor.matmul(
                                sc[:, hp, :],
                                (kts[mo][jc // 8][hp * 64:(hp + 1) * 64,
                                       (jc % 8) * 128:(jc % 8 + 1) * 128]),
                                (qts[mo][Q // 2][hp * 64:(hp + 1) * 64,
                                       (Q % 2) * 512:(Q % 2 + 1) * 512]),
                                start=True,
                                stop=True,
                                skip_group_check=True,
                            )
                        ex = work.tile([128, 2, 512], MM_DT, tag="exp")
                        nc.scalar.activation(ex[:], sc[:], EXP, scale=SCALE)
                        if jc // 4 == Q:  # diagonal chunk: apply causal mask
                            o = jc - 4 * Q
                            for hp in range(2):
                                nc.vector.tensor_mul(
                                    ex[:, hp, :], ex[:, hp, :], maskt[:, o, :]
                                )
                        for hp in range(2):
                            h = 2 * mo + hp
                            nc.tensor.matmul(
                                out_ps[hp][:],
                                (vts[jc][:, h * 65:(h + 1) * 65]),
                                (ex[:, hp, :]),
                                start=(jc == 0),
                                stop=(jc == nchunks - 1),
                                skip_group_check=True,
                            )
                    for hp in range(2):
                        den = rpool.tile([1, 512], F32, tag="den")
                        nc.vector.tensor_copy(den[:], out_ps[hp][64:65, :])
                        rd_f = rpool.tile([1, 512], F32, tag="rdf")
                        nc.vector.reciprocal_approx_fast(out=rd_f[:], in_=den[:])
                        rd = rpool.tile([1, 512], MM_DT, tag="rd")
                        nc.vector.tensor_copy(rd[:], rd_f[:])
                        # broadcast 1/denom across 64 partitions via K=1 matmul
                        rdb = psmm.tile([64, 512], F32, tag="mm")
                        nc.tensor.matmul(
                            rdb[:], (ones64[:]), (rd[:]),
                            start=True, stop=True, skip_group_check=True,
                        )
                        # walrus only accepts fp32r-consumed TensorTensor when
                        # in0 is already fp32r: round both operands via copies
                        att = work.tile([64, 512], MM_DT, tag="att")
                        nc.vector.tensor_copy(att[:], out_ps[hp][0:64, :])
                        rdbs = work.tile([64, 512], MM_DT, tag="rdbs")
                        nc.vector.tensor_copy(rdbs[:], rdb[:])
                        nc.vector.tensor_mul(
                            aT[hp * 64:(hp + 1) * 64, mo, :],
                            att[:],
                            rdbs[:],
                        )

                # out-proj for this query block: partial[s, :] = a @ woT
                for so in range(4):
                    osb = opool.tile([128, D], F32, tag="osb")
                    po = psmm.tile([128, 2, 512], F32, tag="mm")
                    for co in range(2):
                        for nt in range(2):
                            nc.tensor.matmul(
                                po[:, nt, :],
                                (aT[:, co, so * 128:(so + 1) * 128]),
                                (wo[:, co, nt * 512:(nt + 1) * 512]),
                                start=(co == 0),
                                stop=(co == 1),
                                skip_group_check=True,
                            )
                    nc.vector.tensor_copy(
                        osb[:], po.rearrange("p a n -> p (a n)")
                    )
                    nc.sync.dma_start(
                        out_d.rearrange("(a p) n -> p a n", p=128)[:, Q * 4 + so, :],
                        osb[:],
                    )

    nc.compile()
    return nc


_NC = None


def _get_nc():
    global _NC
    if _NC is None:
        _NC = build_bass()
    return _NC


def _causal_mask():
    j = np.arange(128)[:, None, None]
    o = np.arange(4)[None, :, None]
    i = np.arange(512)[None, None, :]
    return ((o * 128 + j) <= i).astype(np.float32)


def kernel(in_features, Wq, Wk, Wv, Wo):
    global LAST_RESULTS
    nc = _get_nc()

    x = np.asarray(in_features, np.float32)
    Wq = np.asarray(Wq, np.float32)
    Wk = np.asarray(Wk, np.float32)
    Wv = np.asarray(Wv, np.float32)
    Wo = np.asarray(Wo, np.float32)
    mask = _causal_mask()

    in_maps = []
    for c in range(N_CORES):
        b, g = divmod(c, 4)
        cols = slice(g * GC, (g + 1) * GC)
        in_maps.append({
            "xT": np.ascontiguousarray(x[b].T),
            "wqT": np.ascontiguousarray(Wq.T[:, cols]),
            "wkT": np.ascontiguousarray(Wk.T[:, cols]),
            "wvT": np.ascontiguousarray(Wv.T[:, cols]),
            "woT": np.ascontiguousarray(Wo[:, cols].T),
            "mask": mask,
        })

    res = bass_utils.run_bass_kernel_spmd(
        nc, in_maps, core_ids=list(range(N_CORES)), trace=TRACE,
    )
    LAST_RESULTS = res
    parts = [res.results[c]["out"] for c in range(N_CORES)]
    out = np.stack([
        parts[4 * b] + parts[4 * b + 1] + parts[4 * b + 2] + parts[4 * b + 3]
        for b in range(B)
    ]).astype(np.float32)
    return out



# revision 5
# speedup vs baseline: 1.6709x; 1.6709x over previous
"""Multi-head self-attention (B=2, S=2048, D=1024, H=16, causal) on 8 TRN2 cores.

Sharding: core c handles batch b=c//4 and head-group g=c%4 (4 heads each).
Host pre-transposes/pre-tiles everything into bf16 so on-chip there are no
transposes and every DMA is contiguous:
  xt   [4,8,128,512]  xt[sb,ko] = x[b].T[ko*128:(ko+1)*128, sb*512:(sb+1)*512]
  wq/wk/wv [128,8,256] w[p,ko,m] = W.T[ko*128+p, g*256+m]
  wo   [128,2,1024]   wo[p,co,n] = Wo[:, g*256+co*128+p].T ... (woT row co*128+p)
  tri  [128,128]      tri[j,i] = (j <= i)  (diagonal-block causal mask)
Host sums the 4 per-group bf16 partial outputs per batch at the end (fp32).

On-chip (all matmuls bf16 into fp32 PSUM, clocked for zero PE idle):
  - qT/kT [128,512] tiles (head-pair channels on partitions), built as the
    xt chunks stream in so the PE starts ~3us into the kernel.
  - v tiles [128, 4(head), 128]: cols 0:64 hold v, cols 64:128 hold 1.0, so
    the PV matmul accumulates the numerator in PSUM rows 0:64 AND the
    softmax denominator replicated across rows 64:128.  Normalization is
    then recip[64,512] + mul[64,512] on DVE only -- no PE broadcast, no
    PE stall (the old 1x64 broadcast matmul stalled the PE ~2.6us per use
    and re-triggered the HAM half-clock throttle).
  - scores computed transposed (scoresT[j,i]) so softmax needs no
    transpose; diagonal 512-col chunks are trimmed to their causal width
    (512-128*o cols) and masked only on the 128-wide triangle block.
  - QK(j+1) is emitted before PV(j) so the exp/mask latency of chunk j
    hides under the QK matmuls of chunk j+1.
  - output projection for block Q is emitted after attention(Q+1, mo=0) so
    its aT operand (produced by the DVE normalization chain) is ready long
    before the PE drains to it.
"""

import os
import sys

sys.path.insert(0, "/opt/trn_rl_repo")
os.environ.setdefault("MYCRO_LOCAL_CACHE", "1")

import numpy as np
import ml_dtypes

import concourse.bacc as bacc
import concourse.bass as bass
import concourse.mybir as mybir
import concourse.tile as tile
from concourse import bass_utils

# The agent image's antenv lacks axon_hooks, so bass_utils' trace path dies on
# import.  Register a shim module that lazily builds the ctypes NTFF hook.
if "antenv.axon_hooks" not in sys.modules:
    import types

    _shim = types.ModuleType("antenv.axon_hooks")
    _shim._HOOK = None

    def _set_hook(hook, _m=_shim):
        _m._HOOK = hook

    def _get_hook(_m=_shim):
        if _m._HOOK is None:
            try:
                from trn_agent_boot.trn_boot import _ntff_profile_via_ctypes

                _m._HOOK = _ntff_profile_via_ctypes("/opt/axon/libaxon_pjrt.so")
            except Exception:
                _m._HOOK = None
        return _m._HOOK

    _shim.set_axon_ntff_profile_hook = _set_hook
    _shim.get_axon_ntff_profile_hook = _get_hook
    sys.modules["antenv.axon_hooks"] = _shim

B, S, D, H = 2, 2048, 1024, 16
DK = 64                      # head dim
HC = 4                       # heads per core
GC = HC * DK                 # 256 cols per head-group
N_CORES = 8
SCALE = 1.0 / np.sqrt(DK)    # 0.125

F32 = mybir.dt.float32
BF16 = mybir.dt.bfloat16
NPBF16 = ml_dtypes.bfloat16

TRACE = False
LAST_RESULTS = None


def build_bass():
    nc = bacc.Bacc("TRN2", target_bir_lowering=False, debug=False)

    DBG = os.environ.get("BASS_DEBUG_DUMP", "") != ""
    xt_d = nc.dram_tensor("xt", [4, 8, 128, 512], BF16, kind="ExternalInput")
    wq_d = nc.dram_tensor("wq", [128, 8, GC], BF16, kind="ExternalInput")
    wk_d = nc.dram_tensor("wk", [128, 8, GC], BF16, kind="ExternalInput")
    wv_d = nc.dram_tensor("wv", [128, 8, GC], BF16, kind="ExternalInput")
    wo_d = nc.dram_tensor("wo", [128, 2, D], BF16, kind="ExternalInput")
    tri_d = nc.dram_tensor("tri", [128, 128], BF16, kind="ExternalInput")
    out_d = nc.dram_tensor("out", [S, D], BF16, kind="ExternalOutput")

    EXP = mybir.ActivationFunctionType.Exp

    with tile.TileContext(nc) as tc:
        with (
            nc.allow_low_precision(reason="bf16 matmuls, fp32 psum accumulate"),
            tc.tile_pool(name="const", bufs=1) as const,
            tc.tile_pool(name="work", bufs=3) as work,
            tc.tile_pool(name="apool", bufs=2) as apool,
            tc.tile_pool(name="opool", bufs=3) as opool,
            tc.tile_pool(name="rpool", bufs=2) as rpool,
            tc.tile_pool(name="psmm", bufs=2, space="PSUM") as psmm,
            tc.tile_pool(name="psout", bufs=4, space="PSUM") as psout,
        ):
            # ---- input DMAs, in consumption order ---------------------------
            wq = const.tile([128, 8, GC], BF16)
            nc.sync.dma_start(wq[:], wq_d[:])
            xts = [[const.tile([128, 512], BF16, name=f"x{sb}{ko}")
                    for ko in range(8)] for sb in range(4)]
            for ko in range(8):
                nc.sync.dma_start(xts[0][ko][:], xt_d[0, ko])
            wk = const.tile([128, 8, GC], BF16)
            nc.sync.dma_start(wk[:], wk_d[:])
            for ko in range(8):
                nc.sync.dma_start(xts[1][ko][:], xt_d[1, ko])
            wv = const.tile([128, 8, GC], BF16)
            nc.sync.dma_start(wv[:], wv_d[:])
            tri = const.tile([128, 128], BF16)
            nc.sync.dma_start(tri[:], tri_d[:])
            for sb in (2, 3):
                for ko in range(8):
                    nc.sync.dma_start(xts[sb][ko][:], xt_d[sb, ko])
            wo = const.tile([128, 2, D], BF16)
            nc.sync.dma_start(wo[:], wo_d[:])

            # v tiles: per j-chunk, per head 64 value cols + 64 ones cols (the
            # ones columns make the PV matmul emit the softmax denominator in
            # PSUM rows 64:128)
            vts = []
            for io in range(16):
                vt = const.tile([128, HC, 128], BF16, name=f"v{io}")
                nc.vector.memset(vt[:, :, 64:128], 1.0)
                vts.append(vt)

            # ---- projections (streamed in xt-chunk order) -------------------
            qts = [[const.tile([128, 512], BF16, name=f"q{m}{s}")
                    for s in range(4)] for m in range(2)]
            kts = [[const.tile([128, 512], BF16, name=f"k{m}{s}")
                    for s in range(4)] for m in range(2)]

            for sb in range(4):
                for w_sb, dst in ((wq, qts), (wk, kts)):
                    for mo in range(2):
                        ps = psmm.tile([128, 2, 512], F32, tag="mm")
                        for ko in range(8):
                            nc.tensor.matmul(
                                ps[:, 0, :],
                                w_sb[:, ko, mo * 128:(mo + 1) * 128],
                                xts[sb][ko][:],
                                start=(ko == 0),
                                stop=(ko == 7),
                                skip_group_check=True,
                            )
                        nc.scalar.copy(dst[mo][sb][:], ps[:, 0, :])
                for i2 in range(4):
                    io = sb * 4 + i2
                    ps = psmm.tile([128, 2, 512], F32, tag="mm")
                    for ko in range(8):
                        nc.tensor.matmul(
                            ps[:, 0, 0:256],
                            xts[sb][ko][:, i2 * 128:(i2 + 1) * 128],
                            wv[:, ko, :],
                            start=(ko == 0),
                            stop=(ko == 7),
                            skip_group_check=True,
                        )
                    nc.vector.tensor_copy(
                        vts[io][:, :, 0:64],
                        ps[:, 0, 0:256].rearrange("p (h e) -> p h e", e=64),
                    )

            # ---- attention + output projection ------------------------------
            def attn(Q, mo, aT):
                n_full = 4 * Q
                nch = n_full + 4
                out_ps = [psout.tile([128, 512], F32, tag="out",
                                     name=f"ops{Q}{mo}{_h}") for _h in range(2)]

                def qk(jc):
                    diag = jc >= n_full
                    o = jc - n_full if diag else 0
                    lo = o * 128 if diag else 0
                    sc = psmm.tile([128, 2, 512], F32, tag="mm")
                    for hp in range(2):
                        nc.tensor.matmul(
                            sc[:, hp, lo:512],
                            kts[mo][jc // 4][hp * 64:(hp + 1) * 64,
                                             (jc % 4) * 128:(jc % 4 + 1) * 128],
                            qts[mo][Q][hp * 64:(hp + 1) * 64, lo:512],
                            start=True,
                            stop=True,
                            skip_group_check=True,
                        )
                    ex = work.tile([128, 2, 512], BF16, tag="exp")
                    nc.scalar.activation(ex[:, :, lo:512], sc[:, :, lo:512],
                                         EXP, scale=SCALE)
                    if diag:
                        for hp in range(2):
                            nc.vector.tensor_mul(
                                ex[:, hp, lo:lo + 128],
                                ex[:, hp, lo:lo + 128],
                                tri[:],
                            )
                    return ex, lo

                def pv(jc, ex, lo):
                    for hp in range(2):
                        nc.tensor.matmul(
                            out_ps[hp][:, lo:512],
                            vts[jc][:, 2 * mo + hp, :],
                            ex[:, hp, lo:512],
                            start=(jc == 0),
                            stop=(jc == nch - 1),
                            skip_group_check=True,
                        )

                pend = qk(0)
                for jc in range(1, nch):
                    nxt = qk(jc)
                    pv(jc - 1, *pend)
                    pend = nxt
                pv(nch - 1, *pend)

                # normalization: rows 64:128 of out_ps hold the denominator
                # replicated 64x, so recip + mul are plain DVE ops.
                for hp in range(2):
                    # plain copy first: custom-DVE ops misread partition-offset
                    # PSUM inputs on HW (sim-only correct), tensor_copy doesn't
                    den = rpool.tile([64, 512], F32, tag="den")
                    nc.vector.tensor_copy(den[:], out_ps[hp][64:128, :])
                    rdb = rpool.tile([64, 512], F32, tag="rd")
                    nc.vector.reciprocal_approx_fast(out=rdb[:], in_=den[:])
                    nc.vector.tensor_mul(
                        aT[hp * 64:(hp + 1) * 64, mo, :],
                        out_ps[hp][0:64, :],
                        rdb[:],
                    )

            def outproj(Q, aT):
                for so in range(4):
                    po = psmm.tile([128, 2, 512], F32, tag="mm")
                    for co in range(2):
                        for nt in range(2):
                            nc.tensor.matmul(
                                po[:, nt, :],
                                aT[:, co, so * 128:(so + 1) * 128],
                                wo[:, co, nt * 512:(nt + 1) * 512],
                                start=(co == 0),
                                stop=(co == 1),
                                skip_group_check=True,
                            )
                    osb = opool.tile([128, D], BF16, tag="osb")
                    nc.scalar.copy(osb[:], po.rearrange("p a n -> p (a n)"))
                    nc.sync.dma_start(
                        out_d.rearrange("(a p) n -> p a n", p=128)[:, Q * 4 + so, :],
                        osb[:],
                    )

            aTs = []
            for Q in range(4):
                aT = apool.tile([128, 2, 512], BF16, tag="aT", name=f"aT{Q}")
                aTs.append(aT)
                attn(Q, 0, aT)
                if Q >= 1:
                    outproj(Q - 1, aTs[Q - 1])
                attn(Q, 1, aT)
                if DBG and Q == 0:
                    dq_d = nc.dram_tensor("dq", [128, 512], BF16,
                                          kind="ExternalOutput")
                    dk_d = nc.dram_tensor("dk", [128, 512], BF16,
                                          kind="ExternalOutput")
                    dv_d = nc.dram_tensor("dv", [128, HC, 128], BF16,
                                          kind="ExternalOutput")
                    da_d = nc.dram_tensor("da", [128, 2, 512], BF16,
                                          kind="ExternalOutput")
                    nc.sync.dma_start(dq_d[:], qts[0][0][:])
                    nc.sync.dma_start(dk_d[:], kts[0][0][:])
                    nc.sync.dma_start(dv_d[:], vts[0][:])
                    nc.sync.dma_start(da_d[:], aT[:])
            outproj(3, aTs[3])

    nc.compile()
    return nc


_NC = None


def _get_nc():
    global _NC
    if _NC is None:
        _NC = build_bass()
    return _NC


def _prep_core_inputs(x, Wq, Wk, Wv, Wo, c):
    b, g = divmod(c, 4)
    cols = slice(g * GC, (g + 1) * GC)
    xT = np.ascontiguousarray(x[b].T).astype(NPBF16)          # [1024, 2048]
    xt = np.ascontiguousarray(
        xT.reshape(8, 128, 4, 512).transpose(2, 0, 1, 3))     # [4,8,128,512]

    def wtile(W):  # W.T[:, cols] -> [128, 8, 256]
        wt = np.ascontiguousarray(W.T[:, cols]).astype(NPBF16)
        return np.ascontiguousarray(wt.reshape(8, 128, GC).transpose(1, 0, 2))

    woT = np.ascontiguousarray(Wo[:, cols].T).astype(NPBF16)  # [256, 1024]
    wo = np.ascontiguousarray(woT.reshape(2, 128, D).transpose(1, 0, 2))
    tri = (np.arange(128)[:, None] <= np.arange(128)[None, :]).astype(NPBF16)
    return {
        "xt": xt,
        "wq": wtile(Wq),
        "wk": wtile(Wk),
        "wv": wtile(Wv),
        "wo": wo,
        "tri": tri,
    }


def kernel(in_features, Wq, Wk, Wv, Wo):
    global LAST_RESULTS
    nc = _get_nc()

    x = np.asarray(in_features, np.float32)
    Wq = np.asarray(Wq, np.float32)
    Wk = np.asarray(Wk, np.float32)
    Wv = np.asarray(Wv, np.float32)
    Wo = np.asarray(Wo, np.float32)

    in_maps = [_prep_core_inputs(x, Wq, Wk, Wv, Wo, c) for c in range(N_CORES)]

    res = bass_utils.run_bass_kernel_spmd(
        nc, in_maps, core_ids=list(range(N_CORES)), trace=TRACE,
    )
    LAST_RESULTS = res
    parts = [np.asarray(res.results[c]["out"], np.float32)
             for c in range(N_CORES)]
    out = np.stack([
        parts[4 * b] + parts[4 * b + 1] + parts[4 * b + 2] + parts[4 * b + 3]
        for b in range(B)
    ]).astype(np.float32)
    return out
